# revision 8
# baseline (speedup 1.0000x reference)
"""DecoderRNN (LSTM + vocab projection) Trainium2 kernel.

Strategy: data-parallel over batch B=64 across 8 NeuronCores (8 examples
per core). Per core:
  1. indirect-DMA gather of caption embeddings (bf16), PE-transpose -> X.T
  2. one GEMM precomputes X @ W_ih.T + b for all 33 cell steps (fp32 out)
  3. 33 sequential LSTM cell steps in transposed layout:
       gates.T = W_hh.T.T @ h.T accumulated into a PSUM [128, 128] tile
       (16 gate-unit chunks x 8 batch columns), gate order packed (g,i,f,o)
       so eltwise runs as a few [128, 32..96] ACT/DVE ops at full partition
       utilization. h.T is written directly into the FC stationary layout.
  4. batched FC GEMM [256, 512] @ [512, 10240] + bias, DMA to out.
All matmuls take bf16 inputs with fp32 PSUM accumulation.
"""

import os
import numpy as np
import ml_dtypes

import concourse.bass as bass
import concourse.tile as tile
from concourse import bacc, mybir
from concourse import bass_utils
from concourse.masks import make_identity

BF16 = ml_dtypes.bfloat16

# Problem shape (hardcoded per the task contract).
B, T, E, H, V = 64, 32, 512, 512, 10000
NCORES = 8
BL = B // NCORES            # 8 examples per core
STEPS = T + 1               # 33 cell steps (features + 32 caption tokens)
FOURH = 4 * H               # 2048
P = 128
NJ = FOURH // P             # 16 gate-unit chunks
NK = H // P                 # 4 contraction chunks
TOKR = T * BL               # 256 token rows (t-major)
ROWS = TOKR + BL            # 264 = tokens + features rows
VP = 10240                  # padded vocab (20 * 512)
NV = VP // 512              # 20 vocab chunks

f32 = mybir.dt.float32
bf16 = mybir.dt.bfloat16
i32 = mybir.dt.int32

# Column permutation packing gates in (g, i, f, o) order.
# PyTorch order along 4H is (i, f, g, o).
_PERM = np.concatenate([
    np.arange(2 * H, 3 * H),   # g
    np.arange(0, H),           # i
    np.arange(H, 2 * H),       # f
    np.arange(3 * H, 4 * H),   # o
])


def _build_program():
    nc = bacc.Bacc(
        "TRN2",
        target_bir_lowering=False,
        debug=False,
        num_devices=NCORES,
    )

    x_feat = nc.dram_tensor("x_feat", [BL, E], bf16, kind="ExternalInput").ap()
    tok_idx = nc.dram_tensor("tok_idx", [TOKR, 1], i32, kind="ExternalInput").ap()
    embed_w = nc.dram_tensor("embed_w", [V, E], bf16, kind="ExternalInput").ap()
    w_ihT = nc.dram_tensor("w_ihT", [E, FOURH], bf16, kind="ExternalInput").ap()
    w_hhT = nc.dram_tensor("w_hhT", [H, FOURH], bf16, kind="ExternalInput").ap()
    bias_t = nc.dram_tensor("bias_t", [P, NJ], f32, kind="ExternalInput").ap()
    fc_wT = nc.dram_tensor("fc_wT", [H, VP], bf16, kind="ExternalInput").ap()
    fc_b_bc = nc.dram_tensor("fc_b_bc", [P, VP], bf16, kind="ExternalInput").ap()
    out = nc.dram_tensor("out", [BL, T, V], f32, kind="ExternalOutput").ap()

    with tile.TileContext(nc) as tc:
        _kernel_body(tc, x_feat, tok_idx, embed_w, w_ihT, w_hhT, bias_t,
                     fc_wT, fc_b_bc, out)

    nc.compile()
    return nc


def _kernel_body(tc, x_feat, tok_idx, embed_w, w_ihT, w_hhT, bias_t,
                 fc_wT, fc_b_bc, out):
    from contextlib import ExitStack
    ctx = ExitStack()
    nc = tc.nc

    # ---- persistent tiles (one bufs=1 pool, distinct tags per name) ----
    cp = ctx.enter_context(tc.tile_pool(name="const", bufs=1))
    wih_sb = cp.tile([P, NK * FOURH], bf16, name="wih_sb", tag="wih_sb")
    whh_sb = cp.tile([P, NK * FOURH], bf16, name="whh_sb", tag="whh_sb")
    fcw_sb = cp.tile([P, NK * VP], bf16, name="fcw_sb", tag="fcw_sb")
    fcb_sb = cp.tile([P, VP], bf16, name="fcb_sb", tag="fcb_sb")
    biast_sb = cp.tile([P, NJ], f32, name="biast_sb", tag="biast_sb")
    ident = cp.tile([P, P], bf16, name="ident", tag="ident")
    idx_sb = cp.tile([P, 2], i32, name="idx_sb", tag="idx_sb")
    xn0 = cp.tile([P, E], bf16, name="xn0", tag="xn0")
    xn1 = cp.tile([P, E], bf16, name="xn1", tag="xn1")
    xf = cp.tile([P, E], bf16, name="xf", tag="xf")
    xT = cp.tile([P, NK * ROWS], bf16, name="xT", tag="xT")
    xpT = cp.tile([P, STEPS * P], f32, name="xpT", tag="xpT")
    hT = cp.tile([P, NK * TOKR], bf16, name="hT", tag="hT")
    h0T = cp.tile([P, NK * BL], bf16, name="h0T", tag="h0T")
    cst = cp.tile([P, NK * BL], f32, name="cst", tag="cst")

    ps = ctx.enter_context(tc.tile_pool(name="ps", bufs=6, space="PSUM"))
    sb = ctx.enter_context(tc.tile_pool(name="sb", bufs=3))

    # ---- load constants ----
    for k in range(NK):
        nc.sync.dma_start(wih_sb[:, k * FOURH:(k + 1) * FOURH],
                          w_ihT[k * P:(k + 1) * P, :])
        nc.sync.dma_start(whh_sb[:, k * FOURH:(k + 1) * FOURH],
                          w_hhT[k * P:(k + 1) * P, :])
        nc.sync.dma_start(fcw_sb[:, k * VP:(k + 1) * VP],
                          fc_wT[k * P:(k + 1) * P, :])
    nc.sync.dma_start(fcb_sb[:], fc_b_bc[:])
    nc.sync.dma_start(biast_sb[:], bias_t[:])
    nc.sync.dma_start(idx_sb[:, 0:1], tok_idx[0:P, :])
    nc.sync.dma_start(idx_sb[:, 1:2], tok_idx[P:2 * P, :])
    nc.sync.dma_start(xf[:BL, :], x_feat[:, :])
    make_identity(nc, ident[:])

    # ---- embedding gather ----
    nc.gpsimd.indirect_dma_start(
        out=xn0[:], out_offset=None, in_=embed_w[:],
        in_offset=bass.IndirectOffsetOnAxis(ap=idx_sb[:, 0:1], axis=0))
    nc.gpsimd.indirect_dma_start(
        out=xn1[:], out_offset=None, in_=embed_w[:],
        in_offset=bass.IndirectOffsetOnAxis(ap=idx_sb[:, 1:2], axis=0))

    # ---- transpose X -> X.T  (token rows 0..255, then feature rows) ----
    for k in range(NK):
        for rc, (src, n_r) in enumerate([(xn0, P), (xn1, P), (xf, BL)]):
            pt = ps.tile([P, 512], bf16, name="pst", tag="ps")
            nc.tensor.transpose(pt[:, :n_r], src[:n_r, k * P:(k + 1) * P],
                                ident[:n_r, :n_r])
            nc.vector.tensor_copy(
                out=xT[:, k * ROWS + rc * P: k * ROWS + rc * P + n_r],
                in_=pt[:, :n_r])

    # ---- X projection GEMM:  xpT[:, c*128 + j*8 + b] = (X @ W_ihT)[row, j*128+p] ----
    xp_view = xpT[:].rearrange("p (s j b) -> p s j b", s=STEPS, j=NJ, b=BL)
    for j in range(NJ):
        pxp = ps.tile([P, 512], f32, name="pxp", tag="ps")
        for k in range(NK):
            nc.tensor.matmul(
                pxp[:, :ROWS],
                lhsT=wih_sb[:, k * FOURH + j * P: k * FOURH + (j + 1) * P],
                rhs=xT[:, k * ROWS:(k + 1) * ROWS],
                start=(k == 0), stop=(k == NK - 1))
        pxp_v = pxp[:, :ROWS].rearrange("p (s b) -> p s b", b=BL)
        # token rows are cells 1..32; feature rows are cell 0
        nc.vector.tensor_scalar_add(
            xp_view[:, 1:STEPS, j, :], pxp_v[:, 0:T, :], biast_sb[:, j:j + 1])
        nc.vector.tensor_scalar_add(
            xp_view[:, 0, j, :], pxp_v[:, T, :], biast_sb[:, j:j + 1])

    # ---- recurrence ----
    hT_view = hT[:].rearrange("p (k s b) -> p k s b", k=NK, s=T, b=BL)
    h0_view = h0T[:].rearrange("p (k b) -> p k b", k=NK)
    c_view = cst[:].rearrange("p (k b) -> p k b", k=NK)

    for c in range(STEPS):
        if c == 0:
            gsrc = xpT[:, 0:P]
        else:
            pg = ps.tile([P, 512], f32, name="pg", tag="ps")
            for j in range(NJ):
                for k in range(NK):
                    if c == 1:
                        rhs = h0T[:, k * BL:(k + 1) * BL]
                    else:
                        off = k * TOKR + (c - 2) * BL
                        rhs = hT[:, off: off + BL]
                    nc.tensor.matmul(
                        pg[:, j * BL:(j + 1) * BL],
                        lhsT=whh_sb[:, k * FOURH + j * P: k * FOURH + (j + 1) * P],
                        rhs=rhs,
                        start=(k == 0), stop=(k == NK - 1))
            gpre = sb.tile([P, P], f32, name="gpre")
            for grp in range(4):
                s = grp * 32
                nc.vector.tensor_add(
                    out=gpre[:, s:s + 32],
                    in0=pg[:, s:s + 32],
                    in1=xpT[:, c * P + s: c * P + s + 32])
            gsrc = gpre[:]

        act_g = sb.tile([P, 32], f32, name="act_g")
        act_if = sb.tile([P, 64], f32, name="act_if")
        act_o = sb.tile([P, 32], f32, name="act_o")
        nc.scalar.activation(act_g[:], gsrc[:, 0:32],
                             mybir.ActivationFunctionType.Tanh)
        nc.scalar.activation(act_if[:], gsrc[:, 32:96],
                             mybir.ActivationFunctionType.Sigmoid)
        nc.scalar.activation(act_o[:], gsrc[:, 96:128],
                             mybir.ActivationFunctionType.Sigmoid)

        if c == 0:
            # c_new = i * g  (previous c is zero)
            nc.vector.tensor_mul(out=cst[:], in0=act_if[:, 0:32], in1=act_g[:])
        else:
            ig = sb.tile([P, 32], f32, name="ig")
            fc2 = sb.tile([P, 32], f32, name="fc2")
            nc.vector.tensor_mul(out=ig[:], in0=act_if[:, 0:32], in1=act_g[:])
            nc.vector.tensor_mul(out=fc2[:], in0=act_if[:, 32:64], in1=cst[:])
            nc.vector.tensor_add(out=cst[:], in0=ig[:], in1=fc2[:])

        tch = sb.tile([P, 32], f32, name="tch")
        nc.scalar.activation(tch[:], cst[:], mybir.ActivationFunctionType.Tanh)

        if c == 0:
            hdst = h0_view
        else:
            hdst = hT_view[:, :, c - 1, :]
        nc.vector.tensor_mul(
            out=hdst,
            in0=act_o[:].rearrange("p (k b) -> p k b", k=NK),
            in1=tch[:].rearrange("p (k b) -> p k b", k=NK))

    # ---- FC GEMM + bias + output DMA ----
    out_v = out[:, :, :]   # [BL, T, V]
    for m in range(TOKR // P):            # 2 row-chunks of 128
        for ng in range(NV // 4):         # 5 groups of 4 vocab chunks
            pts = [ps.tile([P, 512], f32, name="pfc", tag="ps") for _ in range(4)]
            for k in range(NK):
                lhs = hT[:, k * TOKR + m * P: k * TOKR + (m + 1) * P]
                for i in range(4):
                    n = ng * 4 + i
                    nc.tensor.matmul(
                        pts[i],
                        lhsT=lhs,
                        rhs=fcw_sb[:, k * VP + n * 512: k * VP + (n + 1) * 512],
                        start=(k == 0), stop=(k == NK - 1))
            for i in range(4):
                n = ng * 4 + i
                vlo = n * 512
                w = min(V, vlo + 512) - vlo
                if w <= 0:
                    continue
                ob = sb.tile([P, 512], f32, name="ofc")
                nc.vector.tensor_add(out=ob[:], in0=pts[i][:],
                                     in1=fcb_sb[:, vlo:vlo + 512])
                nc.sync.dma_start(
                    out=out_v[:, m * 16:(m + 1) * 16, vlo:vlo + w]
                    .rearrange("b t v -> t b v"),
                    in_=ob[:, :w])
    ctx.close()


_NC_CACHE = {}


def _get_program():
    if "nc" not in _NC_CACHE:
        _NC_CACHE["nc"] = _build_program()
    return _NC_CACHE["nc"]


def make_in_maps(features, captions, embed_W, W_ih, W_hh, b_ih, b_hh, fc_W, fc_b):
    """Host-side sharding + layout prep. Pure layout/dtype work, no math
    beyond summing the two bias vectors."""
    embed_bf = embed_W.astype(BF16)
    w_ihT = np.ascontiguousarray(W_ih.T[:, _PERM]).astype(BF16)
    w_hhT = np.ascontiguousarray(W_hh.T[:, _PERM]).astype(BF16)
    bias = (b_ih + b_hh).astype(np.float32)[_PERM]
    bias_t = np.ascontiguousarray(bias.reshape(NJ, P).T)
    fc_wT = np.zeros((H, VP), dtype=BF16)
    fc_wT[:, :V] = fc_W.T.astype(BF16)
    fcb = np.zeros((VP,), dtype=BF16)
    fcb[:V] = fc_b.astype(BF16)
    fc_b_bc = np.ascontiguousarray(np.broadcast_to(fcb, (P, VP)))

    in_maps = []
    for core in range(NCORES):
        sl = slice(core * BL, (core + 1) * BL)
        cap = captions[sl].astype(np.int32)          # [BL, T]
        tok = np.ascontiguousarray(cap.T).reshape(TOKR, 1)  # t-major
        in_maps.append({
            "x_feat": np.ascontiguousarray(features[sl]).astype(BF16),
            "tok_idx": tok,
            "embed_w": embed_bf,
            "w_ihT": w_ihT,
            "w_hhT": w_hhT,
            "bias_t": bias_t,
            "fc_wT": fc_wT,
            "fc_b_bc": fc_b_bc,
        })
    return in_maps


def _ensure_ntff_hook():
    """The agent image's antenv package lacks axon_hooks; synthesize it so
    run_bass_kernel_spmd(trace=True) can capture NTFF profiles."""
    import sys
    import types
    try:
        from antenv.axon_hooks import get_axon_ntff_profile_hook  # noqa: F401
        return
    except ImportError:
        pass
    import antenv
    mod = types.ModuleType("antenv.axon_hooks")
    state = {}
    mod.set_axon_ntff_profile_hook = lambda h: state.__setitem__("h", h)
    mod.get_axon_ntff_profile_hook = lambda: state.get("h")
    sys.modules["antenv.axon_hooks"] = mod
    antenv.axon_hooks = mod
    try:
        from trn_agent_boot.trn_boot import _ntff_profile_via_ctypes
        hook = _ntff_profile_via_ctypes("/opt/axon/libaxon_pjrt.so")
        if hook is not None:
            mod.set_axon_ntff_profile_hook(hook)
    except Exception as e:  # degrade: tracing skipped, run still works
        print(f"ntff hook setup failed: {e}")


def kernel(features, captions, embed_W, W_ih, W_hh, b_ih, b_hh, fc_W, fc_b,
           _trace=False):
    nc = _get_program()
    in_maps = make_in_maps(features, captions, embed_W, W_ih, W_hh,
                           b_ih, b_hh, fc_W, fc_b)
    if _trace:
        _ensure_ntff_hook()
    res = bass_utils.run_bass_kernel_spmd(
        nc, in_maps, core_ids=list(range(NCORES)), trace=_trace)
    out = np.concatenate([res.results[c]["out"] for c in range(NCORES)], axis=0)
    if _trace:
        kernel.last_result = res
    return out


# revision 10
# speedup vs baseline: 1.1234x; 1.1234x over previous
"""DecoderRNN (LSTM + vocab projection) Trainium2 kernel.

Strategy: data-parallel over batch B=64 across 8 NeuronCores (8 examples
per core). Per core:
  1. indirect-DMA gather of caption embeddings (bf16), PE-transpose -> X.T
  2. one GEMM precomputes X @ W_ih.T + b for all 33 cell steps (fp32 out)
  3. 33 sequential LSTM cell steps in transposed layout:
       gates.T = W_hh.T.T @ h.T accumulated into a PSUM [128, 128] tile
       (16 gate-unit chunks x 8 batch columns), gate order packed (g,i,f,o)
       so eltwise runs as a few [128, 32..96] ACT/DVE ops at full partition
       utilization. h.T is written directly into the FC stationary layout.
  4. batched FC GEMM [256, 512] @ [512, 10240] + bias, DMA to out.
All matmuls take bf16 inputs with fp32 PSUM accumulation.
"""

import os
import numpy as np
import ml_dtypes

import concourse.bass as bass
import concourse.tile as tile
from concourse import bacc, mybir
from concourse import bass_utils
from concourse.masks import make_identity

BF16 = ml_dtypes.bfloat16

# Problem shape (hardcoded per the task contract).
B, T, E, H, V = 64, 32, 512, 512, 10000
NCORES = 8
BL = B // NCORES            # 8 examples per core
STEPS = T + 1               # 33 cell steps (features + 32 caption tokens)
FOURH = 4 * H               # 2048
P = 128
NJ = FOURH // P             # 16 gate-unit chunks
NK = H // P                 # 4 contraction chunks
TOKR = T * BL               # 256 token rows (t-major)
ROWS = TOKR + BL            # 264 = tokens + features rows
VP = 10240                  # padded vocab (20 * 512)
NV = VP // 512              # 20 vocab chunks

f32 = mybir.dt.float32
bf16 = mybir.dt.bfloat16
i32 = mybir.dt.int32

# Column permutation packing gates in (g, i, f, o) order.
# PyTorch order along 4H is (i, f, g, o).
_PERM = np.concatenate([
    np.arange(2 * H, 3 * H),   # g
    np.arange(0, H),           # i
    np.arange(H, 2 * H),       # f
    np.arange(3 * H, 4 * H),   # o
])


def _build_program():
    nc = bacc.Bacc(
        "TRN2",
        target_bir_lowering=False,
        debug=False,
        num_devices=NCORES,
    )

    x_feat = nc.dram_tensor("x_feat", [BL, E], bf16, kind="ExternalInput").ap()
    tok_idx = nc.dram_tensor("tok_idx", [TOKR, 1], i32, kind="ExternalInput").ap()
    embed_w = nc.dram_tensor("embed_w", [V, E], bf16, kind="ExternalInput").ap()
    w_ihT = nc.dram_tensor("w_ihT", [E, FOURH], bf16, kind="ExternalInput").ap()
    w_hhT = nc.dram_tensor("w_hhT", [H, FOURH], bf16, kind="ExternalInput").ap()
    bias_t = nc.dram_tensor("bias_t", [P, NJ], f32, kind="ExternalInput").ap()
    fc_wT = nc.dram_tensor("fc_wT", [H, VP], bf16, kind="ExternalInput").ap()
    fc_b_bc = nc.dram_tensor("fc_b_bc", [P, VP], bf16, kind="ExternalInput").ap()
    out = nc.dram_tensor("out", [BL, T, V], f32, kind="ExternalOutput").ap()

    with tile.TileContext(nc) as tc:
        _kernel_body(tc, x_feat, tok_idx, embed_w, w_ihT, w_hhT, bias_t,
                     fc_wT, fc_b_bc, out)

    nc.compile()
    return nc


def _kernel_body(tc, x_feat, tok_idx, embed_w, w_ihT, w_hhT, bias_t,
                 fc_wT, fc_b_bc, out):
    from contextlib import ExitStack
    ctx = ExitStack()
    nc = tc.nc

    # ---- persistent tiles (one bufs=1 pool, distinct tags per name) ----
    cp = ctx.enter_context(tc.tile_pool(name="const", bufs=1))
    wih_sb = cp.tile([P, NK * FOURH], bf16, name="wih_sb", tag="wih_sb")
    whh_sb = cp.tile([P, NK * FOURH], bf16, name="whh_sb", tag="whh_sb")
    fcw_sb = cp.tile([P, NK * VP], bf16, name="fcw_sb", tag="fcw_sb")
    fcb_sb = cp.tile([P, VP], bf16, name="fcb_sb", tag="fcb_sb")
    biast_sb = cp.tile([P, NJ], f32, name="biast_sb", tag="biast_sb")
    ident = cp.tile([P, P], bf16, name="ident", tag="ident")
    idx_sb = cp.tile([P, 2], i32, name="idx_sb", tag="idx_sb")
    xn0 = cp.tile([P, E], bf16, name="xn0", tag="xn0")
    xn1 = cp.tile([P, E], bf16, name="xn1", tag="xn1")
    xf = cp.tile([P, E], bf16, name="xf", tag="xf")
    xT = cp.tile([P, NK * ROWS], bf16, name="xT", tag="xT")
    xpT = cp.tile([P, STEPS * P], f32, name="xpT", tag="xpT")
    hT = cp.tile([P, NK * TOKR], bf16, name="hT", tag="hT")
    h0T = cp.tile([P, NK * BL], bf16, name="h0T", tag="h0T")
    cst = cp.tile([P, NK * BL], f32, name="cst", tag="cst")

    ps = ctx.enter_context(tc.tile_pool(name="ps", bufs=6, space="PSUM"))
    sb = ctx.enter_context(tc.tile_pool(name="sb", bufs=3))

    # ---- load constants ----
    # Small, critical-path loads first on the sync queue; the big FC weight
    # streams go on the scalar-engine HWDGE queue so they don't block the
    # gather -> transpose -> Xproj -> recurrence critical path.
    nc.sync.dma_start(idx_sb[:, 0:1], tok_idx[0:P, :])
    nc.sync.dma_start(idx_sb[:, 1:2], tok_idx[P:2 * P, :])
    nc.sync.dma_start(xf[:BL, :], x_feat[:, :])
    nc.sync.dma_start(biast_sb[:], bias_t[:])
    for k in range(NK):
        nc.sync.dma_start(wih_sb[:, k * FOURH:(k + 1) * FOURH],
                          w_ihT[k * P:(k + 1) * P, :])
    for k in range(NK):
        nc.sync.dma_start(whh_sb[:, k * FOURH:(k + 1) * FOURH],
                          w_hhT[k * P:(k + 1) * P, :])
    for k in range(NK):
        nc.scalar.dma_start(fcw_sb[:, k * VP:(k + 1) * VP],
                            fc_wT[k * P:(k + 1) * P, :])
    nc.scalar.dma_start(fcb_sb[:], fc_b_bc[:])
    make_identity(nc, ident[:])

    # ---- embedding gather ----
    nc.gpsimd.indirect_dma_start(
        out=xn0[:], out_offset=None, in_=embed_w[:],
        in_offset=bass.IndirectOffsetOnAxis(ap=idx_sb[:, 0:1], axis=0))
    nc.gpsimd.indirect_dma_start(
        out=xn1[:], out_offset=None, in_=embed_w[:],
        in_offset=bass.IndirectOffsetOnAxis(ap=idx_sb[:, 1:2], axis=0))

    # ---- transpose X -> X.T  (token rows 0..255, then feature rows) ----
    for k in range(NK):
        for rc, (src, n_r) in enumerate([(xn0, P), (xn1, P), (xf, BL)]):
            pt = ps.tile([P, 512], bf16, name="pst", tag="ps")
            nc.tensor.transpose(pt[:, :n_r], src[:n_r, k * P:(k + 1) * P],
                                ident[:n_r, :n_r])
            nc.vector.tensor_copy(
                out=xT[:, k * ROWS + rc * P: k * ROWS + rc * P + n_r],
                in_=pt[:, :n_r])

    # ---- X projection GEMM:  xpT[:, c*128 + j*8 + b] = (X @ W_ihT)[row, j*128+p] ----
    xp_view = xpT[:].rearrange("p (s j b) -> p s j b", s=STEPS, j=NJ, b=BL)
    for j in range(NJ):
        pxp = ps.tile([P, 512], f32, name="pxp", tag="ps")
        for k in range(NK):
            nc.tensor.matmul(
                pxp[:, :ROWS],
                lhsT=wih_sb[:, k * FOURH + j * P: k * FOURH + (j + 1) * P],
                rhs=xT[:, k * ROWS:(k + 1) * ROWS],
                start=(k == 0), stop=(k == NK - 1))
        pxp_v = pxp[:, :ROWS].rearrange("p (s b) -> p s b", b=BL)
        # token rows are cells 1..32; feature rows are cell 0
        nc.vector.tensor_scalar_add(
            xp_view[:, 1:STEPS, j, :], pxp_v[:, 0:T, :], biast_sb[:, j:j + 1])
        nc.vector.tensor_scalar_add(
            xp_view[:, 0, j, :], pxp_v[:, T, :], biast_sb[:, j:j + 1])

    # ---- recurrence ----
    hT_view = hT[:].rearrange("p (k s b) -> p k s b", k=NK, s=T, b=BL)
    h0_view = h0T[:].rearrange("p (k b) -> p k b", k=NK)
    c_view = cst[:].rearrange("p (k b) -> p k b", k=NK)

    for c in range(STEPS):
        if c == 0:
            gsrc = xpT[:, 0:P]
        else:
            pg = ps.tile([P, 512], f32, name="pg", tag="ps")
            for j in range(NJ):
                for k in range(NK):
                    if c == 1:
                        rhs = h0T[:, k * BL:(k + 1) * BL]
                    else:
                        off = k * TOKR + (c - 2) * BL
                        rhs = hT[:, off: off + BL]
                    nc.tensor.matmul(
                        pg[:, j * BL:(j + 1) * BL],
                        lhsT=whh_sb[:, k * FOURH + j * P: k * FOURH + (j + 1) * P],
                        rhs=rhs,
                        start=(k == 0), stop=(k == NK - 1))
            gpre = sb.tile([P, P], f32, name="gpre")
            for grp in range(4):
                s = grp * 32
                nc.vector.tensor_add(
                    out=gpre[:, s:s + 32],
                    in0=pg[:, s:s + 32],
                    in1=xpT[:, c * P + s: c * P + s + 32])
            gsrc = gpre[:]

        act_g = sb.tile([P, 32], f32, name="act_g")
        act_if = sb.tile([P, 64], f32, name="act_if")
        act_o = sb.tile([P, 32], f32, name="act_o")
        nc.scalar.activation(act_g[:], gsrc[:, 0:32],
                             mybir.ActivationFunctionType.Tanh)
        nc.scalar.activation(act_if[:], gsrc[:, 32:96],
                             mybir.ActivationFunctionType.Sigmoid)
        nc.scalar.activation(act_o[:], gsrc[:, 96:128],
                             mybir.ActivationFunctionType.Sigmoid)

        if c == 0:
            # c_new = i * g  (previous c is zero)
            nc.vector.tensor_mul(out=cst[:], in0=act_if[:, 0:32], in1=act_g[:])
        else:
            ig = sb.tile([P, 32], f32, name="ig")
            fc2 = sb.tile([P, 32], f32, name="fc2")
            nc.vector.tensor_mul(out=ig[:], in0=act_if[:, 0:32], in1=act_g[:])
            nc.vector.tensor_mul(out=fc2[:], in0=act_if[:, 32:64], in1=cst[:])
            nc.vector.tensor_add(out=cst[:], in0=ig[:], in1=fc2[:])

        tch = sb.tile([P, 32], f32, name="tch")
        nc.scalar.activation(tch[:], cst[:], mybir.ActivationFunctionType.Tanh)

        if c == 0:
            hdst = h0_view
        else:
            hdst = hT_view[:, :, c - 1, :]
        nc.vector.tensor_mul(
            out=hdst,
            in0=act_o[:].rearrange("p (k b) -> p k b", k=NK),
            in1=tch[:].rearrange("p (k b) -> p k b", k=NK))

    # ---- FC GEMM + bias + output DMA ----
    # Per (m, ng): 16 matmuls into 4 PSUM banks, bias-add into a 1 MB
    # staging tile, one DMA per group alternating between the two HWDGE
    # queues so output writeback overlaps compute.
    out_v = out[:, :, :]   # [BL, T, V]
    NGV = 2048             # vocab columns per group
    for m in range(TOKR // P):            # 2 row-chunks of 128
        for ng in range(NV // 4):         # 5 groups of 4 vocab chunks
            pts = [ps.tile([P, 512], f32, name="pfc", tag="ps") for _ in range(4)]
            for k in range(NK):
                lhs = hT[:, k * TOKR + m * P: k * TOKR + (m + 1) * P]
                for i in range(4):
                    n = ng * 4 + i
                    nc.tensor.matmul(
                        pts[i],
                        lhsT=lhs,
                        rhs=fcw_sb[:, k * VP + n * 512: k * VP + (n + 1) * 512],
                        start=(k == 0), stop=(k == NK - 1))
            stg = sb.tile([P, NGV], f32, name="stg", tag="stg")
            for i in range(4):
                vlo = (ng * 4 + i) * 512
                nc.vector.tensor_add(out=stg[:, i * 512:(i + 1) * 512],
                                     in0=pts[i][:],
                                     in1=fcb_sb[:, vlo:vlo + 512])
            glo = ng * NGV
            gw = min(V, glo + NGV) - glo
            eng = nc.sync if (m * 5 + ng) % 2 == 0 else nc.scalar
            eng.dma_start(
                out=out_v[:, m * 16:(m + 1) * 16, glo:glo + gw]
                .rearrange("b t v -> t b v"),
                in_=stg[:, :gw])
    ctx.close()


_NC_CACHE = {}


def _get_program():
    if "nc" not in _NC_CACHE:
        _NC_CACHE["nc"] = _build_program()
    return _NC_CACHE["nc"]


def make_in_maps(features, captions, embed_W, W_ih, W_hh, b_ih, b_hh, fc_W, fc_b):
    """Host-side sharding + layout prep. Pure layout/dtype work, no math
    beyond summing the two bias vectors."""
    embed_bf = embed_W.astype(BF16)
    w_ihT = np.ascontiguousarray(W_ih.T[:, _PERM]).astype(BF16)
    w_hhT = np.ascontiguousarray(W_hh.T[:, _PERM]).astype(BF16)
    bias = (b_ih + b_hh).astype(np.float32)[_PERM]
    bias_t = np.ascontiguousarray(bias.reshape(NJ, P).T)
    fc_wT = np.zeros((H, VP), dtype=BF16)
    fc_wT[:, :V] = fc_W.T.astype(BF16)
    fcb = np.zeros((VP,), dtype=BF16)
    fcb[:V] = fc_b.astype(BF16)
    fc_b_bc = np.ascontiguousarray(np.broadcast_to(fcb, (P, VP)))

    in_maps = []
    for core in range(NCORES):
        sl = slice(core * BL, (core + 1) * BL)
        cap = captions[sl].astype(np.int32)          # [BL, T]
        tok = np.ascontiguousarray(cap.T).reshape(TOKR, 1)  # t-major
        in_maps.append({
            "x_feat": np.ascontiguousarray(features[sl]).astype(BF16),
            "tok_idx": tok,
            "embed_w": embed_bf,
            "w_ihT": w_ihT,
            "w_hhT": w_hhT,
            "bias_t": bias_t,
            "fc_wT": fc_wT,
            "fc_b_bc": fc_b_bc,
        })
    return in_maps


def _ensure_ntff_hook():
    """The agent image's antenv package lacks axon_hooks; synthesize it so
    run_bass_kernel_spmd(trace=True) can capture NTFF profiles."""
    import sys
    import types
    try:
        from antenv.axon_hooks import get_axon_ntff_profile_hook  # noqa: F401
        return
    except ImportError:
        pass
    import antenv
    mod = types.ModuleType("antenv.axon_hooks")
    state = {}
    mod.set_axon_ntff_profile_hook = lambda h: state.__setitem__("h", h)
    mod.get_axon_ntff_profile_hook = lambda: state.get("h")
    sys.modules["antenv.axon_hooks"] = mod
    antenv.axon_hooks = mod
    try:
        from trn_agent_boot.trn_boot import _ntff_profile_via_ctypes
        hook = _ntff_profile_via_ctypes("/opt/axon/libaxon_pjrt.so")
        if hook is not None:
            mod.set_axon_ntff_profile_hook(hook)
    except Exception as e:  # degrade: tracing skipped, run still works
        print(f"ntff hook setup failed: {e}")


def kernel(features, captions, embed_W, W_ih, W_hh, b_ih, b_hh, fc_W, fc_b,
           _trace=False):
    nc = _get_program()
    in_maps = make_in_maps(features, captions, embed_W, W_ih, W_hh,
                           b_ih, b_hh, fc_W, fc_b)
    if _trace:
        _ensure_ntff_hook()
    res = bass_utils.run_bass_kernel_spmd(
        nc, in_maps, core_ids=list(range(NCORES)), trace=_trace)
    out = np.concatenate([res.results[c]["out"] for c in range(NCORES)], axis=0)
    if _trace:
        kernel.last_result = res
    return out


# revision 13
# speedup vs baseline: 1.2737x; 1.1338x over previous
"""DecoderRNN (LSTM + vocab projection) Trainium2 kernel.

Strategy: data-parallel over batch B=64 across 8 NeuronCores (8 examples
per core). Per core:
  1. indirect-DMA gather of caption embeddings (bf16), PE-transpose -> X.T
  2. one GEMM precomputes X @ W_ih.T + b for all 33 cell steps (fp32 out)
  3. 33 sequential LSTM cell steps in transposed layout:
       gates.T = W_hh.T.T @ h.T accumulated into a PSUM [128, 128] tile
       (16 gate-unit chunks x 8 batch columns), gate order packed (g,i,f,o)
       so eltwise runs as a few [128, 32..96] ACT/DVE ops at full partition
       utilization. h.T is written directly into the FC stationary layout.
  4. batched FC GEMM [256, 512] @ [512, 10240] + bias, DMA to out.
All matmuls take bf16 inputs with fp32 PSUM accumulation.
"""

import os
import numpy as np
import ml_dtypes

import concourse.bass as bass
import concourse.tile as tile
from concourse import bacc, mybir
from concourse import bass_utils
from concourse.masks import make_identity

BF16 = ml_dtypes.bfloat16

# Problem shape (hardcoded per the task contract).
B, T, E, H, V = 64, 32, 512, 512, 10000
NCORES = 8
BL = B // NCORES            # 8 examples per core
STEPS = T + 1               # 33 cell steps (features + 32 caption tokens)
FOURH = 4 * H               # 2048
P = 128
NJ = FOURH // P             # 16 gate-unit chunks
NK = H // P                 # 4 contraction chunks
TOKR = T * BL               # 256 token rows (t-major)
ROWS = TOKR + BL            # 264 = tokens + features rows
VP = 10240                  # padded vocab (20 * 512)
NV = VP // 512              # 20 vocab chunks

f32 = mybir.dt.float32
bf16 = mybir.dt.bfloat16
i32 = mybir.dt.int32

# Column permutation packing gates in (g, i, f, o) order.
# PyTorch order along 4H is (i, f, g, o).
_PERM = np.concatenate([
    np.arange(2 * H, 3 * H),   # g
    np.arange(0, H),           # i
    np.arange(H, 2 * H),       # f
    np.arange(3 * H, 4 * H),   # o
])


def _build_program():
    nc = bacc.Bacc(
        "TRN2",
        target_bir_lowering=False,
        debug=False,
        num_devices=NCORES,
    )

    x_feat = nc.dram_tensor("x_feat", [BL, E], bf16, kind="ExternalInput").ap()
    tok_idx = nc.dram_tensor("tok_idx", [TOKR, 1], i32, kind="ExternalInput").ap()
    embed_w = nc.dram_tensor("embed_w", [V, E], bf16, kind="ExternalInput").ap()
    w_ihT = nc.dram_tensor("w_ihT", [E, FOURH], bf16, kind="ExternalInput").ap()
    w_hhT = nc.dram_tensor("w_hhT", [H, FOURH], bf16, kind="ExternalInput").ap()
    bias_t = nc.dram_tensor("bias_t", [P, NJ], f32, kind="ExternalInput").ap()
    fc_wT = nc.dram_tensor("fc_wT", [H, VP], bf16, kind="ExternalInput").ap()
    fc_b_bc = nc.dram_tensor("fc_b_bc", [P, VP], bf16, kind="ExternalInput").ap()
    out = nc.dram_tensor("out", [BL, T, V], f32, kind="ExternalOutput").ap()

    with tile.TileContext(nc) as tc:
        _kernel_body(tc, x_feat, tok_idx, embed_w, w_ihT, w_hhT, bias_t,
                     fc_wT, fc_b_bc, out)

    nc.compile()
    return nc


def _kernel_body(tc, x_feat, tok_idx, embed_w, w_ihT, w_hhT, bias_t,
                 fc_wT, fc_b_bc, out):
    from contextlib import ExitStack
    ctx = ExitStack()
    nc = tc.nc

    # ---- persistent tiles (one bufs=1 pool, distinct tags per name) ----
    cp = ctx.enter_context(tc.tile_pool(name="const", bufs=1))
    wih_sb = cp.tile([P, NK * FOURH], bf16, name="wih_sb", tag="wih_sb")
    whh_sb = cp.tile([P, NK * FOURH], bf16, name="whh_sb", tag="whh_sb")
    fcw_sb = cp.tile([P, NK * VP], bf16, name="fcw_sb", tag="fcw_sb")
    fcb_sb = cp.tile([P, VP], bf16, name="fcb_sb", tag="fcb_sb")
    biast_sb = cp.tile([P, NJ], f32, name="biast_sb", tag="biast_sb")
    ident = cp.tile([P, P], bf16, name="ident", tag="ident")
    idx_sb = cp.tile([P, 2], i32, name="idx_sb", tag="idx_sb")
    xn0 = cp.tile([P, E], bf16, name="xn0", tag="xn0")
    xn1 = cp.tile([P, E], bf16, name="xn1", tag="xn1")
    xf = cp.tile([P, E], bf16, name="xf", tag="xf")
    xT = cp.tile([P, NK * ROWS], bf16, name="xT", tag="xT")
    xpT = cp.tile([P, STEPS * P], bf16, name="xpT", tag="xpT")
    hT = cp.tile([P, NK * TOKR], bf16, name="hT", tag="hT")
    h0T = cp.tile([P, NK * BL], bf16, name="h0T", tag="h0T")
    cst = cp.tile([P, NK * BL], f32, name="cst", tag="cst")

    ps = ctx.enter_context(tc.tile_pool(name="ps", bufs=2, space="PSUM"))
    sb = ctx.enter_context(tc.tile_pool(name="sb", bufs=3))

    # ---- load constants ----
    # Small, critical-path loads first on the sync queue; the big FC weight
    # streams go on the scalar-engine HWDGE queue so they don't block the
    # gather -> transpose -> Xproj -> recurrence critical path.
    nc.sync.dma_start(idx_sb[:].rearrange("p (c o) -> p c o", o=1),
                      tok_idx.rearrange("(c p) o -> p c o", p=P))
    nc.sync.dma_start(xf[:BL, :], x_feat[:, :])
    nc.sync.dma_start(wih_sb[:].rearrange("p (k f) -> p k f", k=NK),
                      w_ihT.rearrange("(k p) f -> p k f", p=P))
    nc.sync.dma_start(biast_sb[:], bias_t[:])
    nc.sync.dma_start(whh_sb[:].rearrange("p (k f) -> p k f", k=NK),
                      w_hhT.rearrange("(k p) f -> p k f", p=P))
    nc.scalar.dma_start(fcw_sb[:].rearrange("p (k f) -> p k f", k=NK),
                        fc_wT.rearrange("(k p) f -> p k f", p=P))
    nc.scalar.dma_start(fcb_sb[:], fc_b_bc[:])
    make_identity(nc, ident[:])

    # ---- embedding gather ----
    nc.gpsimd.indirect_dma_start(
        out=xn0[:], out_offset=None, in_=embed_w[:],
        in_offset=bass.IndirectOffsetOnAxis(ap=idx_sb[:, 0:1], axis=0))
    nc.gpsimd.indirect_dma_start(
        out=xn1[:], out_offset=None, in_=embed_w[:],
        in_offset=bass.IndirectOffsetOnAxis(ap=idx_sb[:, 1:2], axis=0))

    # ---- transpose X -> X.T  (token rows 0..255, then feature rows) ----
    for k in range(NK):
        for rc, (src, n_r) in enumerate([(xn0, P), (xn1, P), (xf, BL)]):
            pt = ps.tile([P, 512], bf16, name="pst", tag="ps")
            nc.tensor.transpose(pt[:, :n_r], src[:n_r, k * P:(k + 1) * P],
                                ident[:n_r, :n_r])
            nc.vector.tensor_copy(
                out=xT[:, k * ROWS + rc * P: k * ROWS + rc * P + n_r],
                in_=pt[:, :n_r])

    # ---- X projection GEMM:  xpT[:, c*128 + j*8 + b] = (X @ W_ihT)[row, j*128+p] ----
    xp_view = xpT[:].rearrange("p (s j b) -> p s j b", s=STEPS, j=NJ, b=BL)
    for j in range(NJ):
        pxp = ps.tile([P, 512], f32, name="pxp", tag="ps")
        for k in range(NK):
            nc.tensor.matmul(
                pxp[:, :ROWS],
                lhsT=wih_sb[:, k * FOURH + j * P: k * FOURH + (j + 1) * P],
                rhs=xT[:, k * ROWS:(k + 1) * ROWS],
                start=(k == 0), stop=(k == NK - 1))
        pxp_v = pxp[:, :ROWS].rearrange("p (s b) -> p s b", b=BL)
        # token rows are cells 1..32; feature rows are cell 0
        nc.vector.tensor_scalar_add(
            xp_view[:, 1:STEPS, j, :], pxp_v[:, 0:T, :], biast_sb[:, j:j + 1])
        nc.vector.tensor_scalar_add(
            xp_view[:, 0, j, :], pxp_v[:, T, :], biast_sb[:, j:j + 1])

    # ---- recurrence ----
    hT_view = hT[:].rearrange("p (k s b) -> p k s b", k=NK, s=T, b=BL)
    h0_view = h0T[:].rearrange("p (k b) -> p k b", k=NK)
    c_view = cst[:].rearrange("p (k b) -> p k b", k=NK)

    def _hprev(c, k):
        if c == 1:
            return h0T[:, k * BL:(k + 1) * BL]
        off = k * TOKR + (c - 2) * BL
        return hT[:, off: off + BL]

    # Gate groups: (name, j-range, xp column offset, width)
    GRP = (("gg", 0, 4, 0, 32), ("gif", 4, 12, 32, 64), ("go", 12, 16, 96, 32))

    for c in range(STEPS):
        if c == 0:
            g_g, g_if, g_o = (xpT[:, 0:32], xpT[:, 32:96], xpT[:, 96:128])
        else:
            tiles = {}
            for (tag, j0, j1, xoff, wdt) in GRP:
                pg = ps.tile([P, 64], f32, name=tag, tag=tag)[:, :wdt]
                tiles[tag] = pg
                # identity matmul preloads PSUM with the X-projection term
                # (start=True sets has_written so W matmuls accumulate)
                nc.tensor.matmul(
                    pg, lhsT=ident[:], rhs=xpT[:, c * P + xoff: c * P + xoff + wdt],
                    start=True, stop=False, skip_group_check=True)
                for j in range(j0, j1):
                    for k in range(NK):
                        nc.tensor.matmul(
                            pg[:, (j - j0) * BL:(j - j0 + 1) * BL],
                            lhsT=whh_sb[:, k * FOURH + j * P: k * FOURH + (j + 1) * P],
                            rhs=_hprev(c, k),
                            start=False, stop=(j == j1 - 1 and k == NK - 1),
                            skip_group_check=True)
            g_g, g_if, g_o = tiles["gg"], tiles["gif"], tiles["go"]

        act_g = sb.tile([P, 32], f32, name="act_g")
        act_if = sb.tile([P, 64], f32, name="act_if")
        act_o = sb.tile([P, 32], f32, name="act_o")
        nc.scalar.activation(act_g[:], g_g,
                             mybir.ActivationFunctionType.Tanh)
        nc.scalar.activation(act_if[:], g_if,
                             mybir.ActivationFunctionType.Sigmoid)
        nc.scalar.activation(act_o[:], g_o,
                             mybir.ActivationFunctionType.Sigmoid)

        if c == 0:
            # c_new = i * g  (previous c is zero)
            nc.vector.tensor_mul(out=cst[:], in0=act_if[:, 0:32], in1=act_g[:])
        else:
            ig = sb.tile([P, 32], f32, name="ig")
            fc2 = sb.tile([P, 32], f32, name="fc2")
            nc.vector.tensor_mul(out=ig[:], in0=act_if[:, 0:32], in1=act_g[:])
            nc.vector.tensor_mul(out=fc2[:], in0=act_if[:, 32:64], in1=cst[:])
            nc.vector.tensor_add(out=cst[:], in0=ig[:], in1=fc2[:])

        tch = sb.tile([P, 32], f32, name="tch")
        nc.scalar.activation(tch[:], cst[:], mybir.ActivationFunctionType.Tanh)

        if c == 0:
            hdst = h0_view
        else:
            hdst = hT_view[:, :, c - 1, :]
        nc.vector.tensor_mul(
            out=hdst,
            in0=act_o[:].rearrange("p (k b) -> p k b", k=NK),
            in1=tch[:].rearrange("p (k b) -> p k b", k=NK))

    # ---- FC GEMM + bias + output DMA ----
    # Per (m, ng): 16 matmuls into 4 PSUM banks, bias-add into a 1 MB
    # staging tile, one DMA per group alternating between the two HWDGE
    # queues so output writeback overlaps compute.
    out_v = out[:, :, :]   # [BL, T, V]
    NGV = 2048             # vocab columns per group
    for m in range(TOKR // P):            # 2 row-chunks of 128
        for ng in range(NV // 4):         # 5 groups of 4 vocab chunks
            _ftags = ("gg", "gif", "go", "ps")
            pts = [ps.tile([P, 512], f32, name="pfc", tag=_ftags[i])
                   for i in range(4)]
            for k in range(NK):
                lhs = hT[:, k * TOKR + m * P: k * TOKR + (m + 1) * P]
                for i in range(4):
                    n = ng * 4 + i
                    nc.tensor.matmul(
                        pts[i],
                        lhsT=lhs,
                        rhs=fcw_sb[:, k * VP + n * 512: k * VP + (n + 1) * 512],
                        start=(k == 0), stop=(k == NK - 1))
            stg = sb.tile([P, NGV], f32, name="stg", tag="stg")
            for i in range(4):
                vlo = (ng * 4 + i) * 512
                nc.vector.tensor_add(out=stg[:, i * 512:(i + 1) * 512],
                                     in0=pts[i][:],
                                     in1=fcb_sb[:, vlo:vlo + 512])
            glo = ng * NGV
            gw = min(V, glo + NGV) - glo
            eng = nc.sync if (m * 5 + ng) % 2 == 0 else nc.scalar
            eng.dma_start(
                out=out_v[:, m * 16:(m + 1) * 16, glo:glo + gw]
                .rearrange("b t v -> t b v"),
                in_=stg[:, :gw])
    ctx.close()


_NC_CACHE = {}


def _get_program():
    if "nc" not in _NC_CACHE:
        _NC_CACHE["nc"] = _build_program()
    return _NC_CACHE["nc"]


def make_in_maps(features, captions, embed_W, W_ih, W_hh, b_ih, b_hh, fc_W, fc_b):
    """Host-side sharding + layout prep. Pure layout/dtype work, no math
    beyond summing the two bias vectors."""
    embed_bf = embed_W.astype(BF16)
    w_ihT = np.ascontiguousarray(W_ih.T[:, _PERM]).astype(BF16)
    w_hhT = np.ascontiguousarray(W_hh.T[:, _PERM]).astype(BF16)
    bias = (b_ih + b_hh).astype(np.float32)[_PERM]
    bias_t = np.ascontiguousarray(bias.reshape(NJ, P).T)
    fc_wT = np.zeros((H, VP), dtype=BF16)
    fc_wT[:, :V] = fc_W.T.astype(BF16)
    fcb = np.zeros((VP,), dtype=BF16)
    fcb[:V] = fc_b.astype(BF16)
    fc_b_bc = np.ascontiguousarray(np.broadcast_to(fcb, (P, VP)))

    in_maps = []
    for core in range(NCORES):
        sl = slice(core * BL, (core + 1) * BL)
        cap = captions[sl].astype(np.int32)          # [BL, T]
        tok = np.ascontiguousarray(cap.T).reshape(TOKR, 1)  # t-major
        in_maps.append({
            "x_feat": np.ascontiguousarray(features[sl]).astype(BF16),
            "tok_idx": tok,
            "embed_w": embed_bf,
            "w_ihT": w_ihT,
            "w_hhT": w_hhT,
            "bias_t": bias_t,
            "fc_wT": fc_wT,
            "fc_b_bc": fc_b_bc,
        })
    return in_maps


def _ensure_ntff_hook():
    """The agent image's antenv package lacks axon_hooks; synthesize it so
    run_bass_kernel_spmd(trace=True) can capture NTFF profiles."""
    import sys
    import types
    try:
        from antenv.axon_hooks import get_axon_ntff_profile_hook  # noqa: F401
        return
    except ImportError:
        pass
    import antenv
    mod = types.ModuleType("antenv.axon_hooks")
    state = {}
    mod.set_axon_ntff_profile_hook = lambda h: state.__setitem__("h", h)
    mod.get_axon_ntff_profile_hook = lambda: state.get("h")
    sys.modules["antenv.axon_hooks"] = mod
    antenv.axon_hooks = mod
    try:
        from trn_agent_boot.trn_boot import _ntff_profile_via_ctypes
        hook = _ntff_profile_via_ctypes("/opt/axon/libaxon_pjrt.so")
        if hook is not None:
            mod.set_axon_ntff_profile_hook(hook)
    except Exception as e:  # degrade: tracing skipped, run still works
        print(f"ntff hook setup failed: {e}")


def kernel(features, captions, embed_W, W_ih, W_hh, b_ih, b_hh, fc_W, fc_b,
           _trace=False):
    nc = _get_program()
    in_maps = make_in_maps(features, captions, embed_W, W_ih, W_hh,
                           b_ih, b_hh, fc_W, fc_b)
    if _trace:
        _ensure_ntff_hook()
    res = bass_utils.run_bass_kernel_spmd(
        nc, in_maps, core_ids=list(range(NCORES)), trace=_trace)
    out = np.concatenate([res.results[c]["out"] for c in range(NCORES)], axis=0)
    if _trace:
        kernel.last_result = res
    return out
